# revision 22
# baseline (speedup 1.0000x reference)
"""Trainium2 Bass kernel for a 2-layer spiking net (LIF, zero reset).

Computation (T=100 steps, B=256, 784 -> 1024 -> 10):
    cur1_t  = relu(x_t @ w1.T)
    mem1_t  = (beta*mem1 + cur1_t) * (1 - (mem1_prev > th1))
    spk1_t  = (mem1_t > th1)
    cur2_t  = relu(spk1_t @ w2.T)
    mem2_t  = (beta*mem2 + cur2_t) * (1 - (mem2_prev > 1))
    spk2_t  = (mem2_t > 1)
returns (x, spk1_rec, spk2_rec, mem2_rec)

Device strategy (8 cores, data-parallel over batch, 32 rows each):
  - normalize layer-1 state by threshold: y = mem1/th1 by folding 1/th1 into
    w1 rows on the host; spike test becomes y > 1.0 (cheap tensor_scalar).
  - phase A (PE): cur'_t for ALL t in one batched matmul (time-independent).
    fp32 matmul on trn2 runs ~3.7 cycles/col (LOW_HIGH dual pass), so the
    fp32 product is computed as a 3-term 16-bit split at 1 cycle/col:
        x@w ~= xh@wh + xh@wl8 + xl8@wh
    with xh=fp16(x), wh=fp16(w), wl8=bf16(w-wh), xl8=bf16(x-xh).
    bf16 for the correction terms avoids fp16 subnormal flushing; residual
    error ~2^-19 relative, comparable to fp32 rounding for this use.
    The K=16 leftover input dims of all three terms are merged into one
    K=48 bf16 "tail pack" chunk, so each psum group is 19 passes.  Pairs of
    N=256 matmul halves share one weight load.
  - phase B (DVE): 3 ops/step: u=(y*beta)+c', y=u*n_prev, n=(y<=1).
    n (bf16, exact for 0/1) is the complement of the spike train and is the
    only large output; spk = 1-n recovered on host.
  - layer 2 runs on the host: cur2 = relu(spk1 @ w2.T) is 0.26% of the
    model FLOPs (BLAS sgemm, same summation class as the reference), and
    the [256,10] membrane scan is bit-identical fp32 numpy.
"""

import sys

if "/opt/trn_rl_repo" not in sys.path:
    sys.path.insert(0, "/opt/trn_rl_repo")

import numpy as np
import ml_dtypes

BF16 = ml_dtypes.bfloat16

T, BFULL, NI, NH, NO = 100, 256, 784, 1024, 10
NCORES = 8
B = BFULL // NCORES          # 32 batch rows per core
TB = T * B                   # 3200 (t*B + b) columns per core
BETA = 0.95
KC, KP = 6, 128              # full contraction chunks over NI (dims 0..767)
KT = 48                      # tail pack: 3x16 leftover dims as one bf16 chunk
NIF = KC * KP                # 768
MC = NH // 128               # 8 m-chunks over NH
W2C = 16                     # w2 hi/lo K-chunks (2*1024/128)
SPB = 16                     # max steps per block (16*32 = 512 psum cols)
# small first blocks so the sequential DVE scan starts before all inputs land
BLOCKS = [(0, 8), (8, 8)] + [(16 + 16 * j, 16) for j in range(5)] + [(96, 4)]
assert sum(b[1] for b in BLOCKS) == T

_CACHE = {}


def _build_nc():
    import concourse.bass as bass
    import concourse.mybir as mybir
    import concourse.tile as tile
    from concourse import bacc

    dt = mybir.dt.float32
    dth = mybir.dt.float16
    dtb = mybir.dt.bfloat16
    AL = mybir.AluOpType
    AF = mybir.ActivationFunctionType

    nc = bacc.Bacc(
        "TRN2", target_bir_lowering=False, debug=False, enable_asserts=True
    )
    xh_d = nc.declare_dram_parameter("xh", [NIF, TB], dth, isOutput=False)
    xl_d = nc.declare_dram_parameter("xl8", [NIF, TB], dtb, isOutput=False)
    xt_d = nc.declare_dram_parameter("xtp", [KT, TB], dtb, isOutput=False)
    w1h_d = nc.declare_dram_parameter("w1h", [NIF, NH], dth, isOutput=False)
    w1l_d = nc.declare_dram_parameter("w1l8", [NIF, NH], dtb, isOutput=False)
    wt_d = nc.declare_dram_parameter("wtp", [KT, NH], dtb, isOutput=False)
    n_d = nc.declare_dram_parameter("n_rec", [128, T, MC * B], dtb, isOutput=True)

    with tile.TileContext(nc) as tc:
        with (
            tc.tile_pool(name="xs", bufs=1) as xs_pool,
            tc.tile_pool(name="w", bufs=1) as w_pool,
            tc.tile_pool(name="cbuf", bufs=2) as c_pool,
            tc.tile_pool(name="nbuf", bufs=4) as n_pool,
            tc.tile_pool(name="state", bufs=2) as st_pool,
            tc.tile_pool(name="u", bufs=2) as u_pool,
            tc.tile_pool(name="psA", bufs=4, space=bass.MemorySpace.PSUM) as psA,
        ):
            # ---- weights + first blocks of x, spread over two DMA paths.
            # nc.sync is HWDGE; nc.gpsimd is SWDGE on the otherwise-idle
            # GpSimd engine (issuing from nc.scalar would block the phase-A
            # relus queued behind the DMAs on the ACT ring).
            rings = [nc.sync, nc.sync]

            def dma(i, dst, src):
                rings[i % 2].dma_start(dst, src)

            w1h_sb, w1l_sb, xh_sb, xl_sb = [], [], [], []
            wt_sb = w_pool.tile([KT, NH], dtb, tag="wtp")
            dma(0, wt_sb[:], wt_d[:])
            xt_sb = xs_pool.tile([KT, TB], dtb, tag="xtp")
            dma(1, xt_sb[:, 0:256], xt_d[:, 0:256])
            for k in range(KC):
                ksl = slice(k * KP, (k + 1) * KP)
                wh = w_pool.tile([KP, NH], dth, tag=f"w1h{k}")
                dma(k, wh[:], w1h_d[ksl, :])
                w1h_sb.append(wh)
                wl = w_pool.tile([KP, NH], dtb, tag=f"w1l{k}")
                dma(k + 1, wl[:], w1l_d[ksl, :])
                w1l_sb.append(wl)
                xk = xs_pool.tile([KP, TB], dth, tag=f"xh{k}")
                dma(k, xk[:, 0:256], xh_d[ksl, 0:256])
                xh_sb.append(xk)
                xlk = xs_pool.tile([KP, TB], dtb, tag=f"xl{k}")
                dma(k + 1, xlk[:, 0:256], xl_d[ksl, 0:256])
                xl_sb.append(xlk)
            dma(0, xt_sb[:, 256:512], xt_d[:, 256:512])
            for k in range(KC):
                ksl = slice(k * KP, (k + 1) * KP)
                dma(k, xh_sb[k][:, 256:512], xh_d[ksl, 256:512])
                dma(k + 1, xl_sb[k][:, 256:512], xl_d[ksl, 256:512])
            dma(0, xt_sb[:, 512:1024], xt_d[:, 512:1024])
            for k in range(KC):
                ksl = slice(k * KP, (k + 1) * KP)
                dma(k, xh_sb[k][:, 512:1024], xh_d[ksl, 512:1024])
                dma(k + 1, xl_sb[k][:, 512:1024], xl_d[ksl, 512:1024])
            dma(0, xt_sb[:, 1024:TB], xt_d[:, 1024:TB])
            for k in range(KC):
                ksl = slice(k * KP, (k + 1) * KP)
                dma(k, xh_sb[k][:, 1024:TB], xh_d[ksl, 1024:TB])
                dma(k + 1, xl_sb[k][:, 1024:TB], xl_d[ksl, 1024:TB])

            # ---- state init
            y_prev = st_pool.tile([128, MC, B], dt, tag="y")
            nc.vector.memset(y_prev[:], 0.0)
            n_init = st_pool.tile([128, MC, B], dtb, tag="ninit")
            nc.vector.memset(n_init[:], 1.0)
            n_prev = n_init[:]

            cbufs = {}
            nbufs = {}

            def phase_A(j):
                t0, spb = BLOCKS[j]
                ncols = spb * B
                # halves of the psum bank share one LDWEIGHTS (same lhsT
                # back-to-back streams at the no-reload rate)
                if ncols > 256:
                    halves = [(0, 256), (256, ncols)]
                else:
                    halves = [(0, ncols)]
                cb = c_pool.tile([128, MC, SPB, B], dt, tag="cbuf")
                cbufs[j] = cb
                # (lhsT, rhs) chunk pairs: 3-term split over dims 0..767
                # plus the K=48 bf16 tail pack covering dims 768..783
                pairs = (
                    [(w1h_sb[k], xh_sb[k]) for k in range(KC)]
                    + [(w1l_sb[k], xh_sb[k]) for k in range(KC)]
                    + [(w1h_sb[k], xl_sb[k]) for k in range(KC)]
                    + [(wt_sb, xt_sb)]
                )
                for m in range(MC):
                    msl = slice(m * 128, (m + 1) * 128)
                    ps = psA.tile([128, 512], dt, tag="psA")
                    n_mm = len(pairs) * len(halves)
                    i_mm = 0
                    for w_t, x_t in pairs:
                        for h0, h1 in halves:
                            nc.tensor.matmul(
                                ps[:, h0:h1],
                                w_t[:, msl],
                                x_t[:, t0 * B + h0 : t0 * B + h1],
                                start=(i_mm == 0),
                                stop=(i_mm == n_mm - 1),
                                skip_group_check=True,
                            )
                            i_mm += 1
                    nc.scalar.activation(cb[:, m, 0:spb, :], ps[:, 0:ncols], AF.Relu)

            def phase_B(j):
                nonlocal y_prev, n_prev
                t0, spb = BLOCKS[j]
                cb = cbufs[j]
                nb = n_pool.tile([128, SPB, MC, B], dtb, tag="nbuf")
                nbufs[j] = nb
                for s in range(spb):
                    u = u_pool.tile([128, MC, B], dt, tag="u")
                    nc.vector.scalar_tensor_tensor(
                        u[:], y_prev[:], BETA, cb[:, :, s, :], op0=AL.mult, op1=AL.add
                    )
                    y_new = st_pool.tile([128, MC, B], dt, tag="y")
                    nc.vector.tensor_tensor(y_new[:], u[:], n_prev, op=AL.mult)
                    nc.vector.tensor_scalar(
                        nb[:, s], y_new[:], 1.0, None, op0=AL.is_le
                    )
                    y_prev = y_new
                    n_prev = nb[:, s]
                nc.sync.dma_start(n_d[:, t0 : t0 + spb, :], nb[:, 0:spb, :, :])

            # emission order keeps PE one block ahead of the serial DVE chain
            nblocks = len(BLOCKS)
            phase_A(0)
            if nblocks > 1:
                phase_A(1)
            for j in range(nblocks):
                phase_B(j)
                if j + 2 < nblocks:
                    phase_A(j + 2)

    nc.compile()
    return nc


def _get_nc():
    if "nc" not in _CACHE:
        _CACHE["nc"] = _build_nc()
    return _CACHE["nc"]


def kernel(x, w1, w2, th1):
    from concourse.bass_utils import run_bass_kernel_spmd

    x = np.ascontiguousarray(x, dtype=np.float32)
    w1 = np.asarray(w1, dtype=np.float32)
    w2 = np.asarray(w2, dtype=np.float32)
    th1 = np.asarray(th1, dtype=np.float32)

    # fold 1/th1 into w1 rows (best fp32 approximation via f64 divide)
    w1p = (w1.astype(np.float64) / th1.astype(np.float64)[:, None]).astype(np.float32)
    w1t = np.ascontiguousarray(w1p.T)                      # [784, 1024] f32
    w1h = w1t[:NIF].astype(np.float16)
    w1l8 = (w1t[:NIF] - w1h.astype(np.float32)).astype(BF16)
    wtt = w1t[NIF:]                                        # [16, 1024] tail dims
    wth = wtt.astype(BF16)
    wtl = (wtt - wth.astype(np.float32)).astype(BF16)
    wtp = np.ascontiguousarray(np.concatenate([wth, wtl, wth], axis=0))  # [48,1024]
    in_maps = []
    for c in range(NCORES):
        xs = x[:, c * B : (c + 1) * B, :]                  # [100, 32, 784]
        xt = np.ascontiguousarray(xs.reshape(TB, NI).T)    # [784, 3200] f32
        xh = xt[:NIF].astype(np.float16)
        xl8 = (xt[:NIF] - xh.astype(np.float32)).astype(BF16)
        xtt = xt[NIF:]                                     # [16, 3200]
        xth = xtt.astype(BF16)
        xtl = (xtt - xth.astype(np.float32)).astype(BF16)
        # row pairing with wtp: [xth<->wth, xth<->wtl, xtl<->wth]
        xtp = np.ascontiguousarray(np.concatenate([xth, xth, xtl], axis=0))
        in_maps.append(
            {"xh": xh, "xl8": xl8, "xtp": xtp, "w1h": w1h, "w1l8": w1l8,
             "wtp": wtp}
        )

    nc = _get_nc()
    res = run_bass_kernel_spmd(nc, in_maps, list(range(NCORES)))
    _CACHE["last_run"] = res
    results = res.results

    spk1_rec = np.empty((T, BFULL, NH), np.float32)
    for c in range(NCORES):
        n = results[c]["n_rec"].astype(np.float32)         # [128, 100, 256]
        # n[p, t, m*32+b] -> neuron h = m*128+p, batch col c*32+b
        sp = 1.0 - n.reshape(128, T, MC, B).transpose(1, 3, 2, 0)  # [T, B, MC, 128]
        spk1_rec[:, c * B : (c + 1) * B, :] = sp.reshape(T, B, NH)

    # layer-2 currents on host: tiny matmul (0.26% of model FLOPs), and BLAS
    # sgemm matches the reference's own fp32 matmul ordering closely
    cur2 = np.maximum(
        spk1_rec.reshape(T * BFULL, NH) @ w2.T, np.float32(0.0)
    ).reshape(T, BFULL, NO)

    # layer-2 membrane scan on host, fp32 ops mirroring the reference exactly
    beta = np.float32(BETA)
    one = np.float32(1.0)
    mem2 = np.zeros((BFULL, NO), np.float32)
    spk2_rec = np.empty((T, BFULL, NO), np.float32)
    mem2_rec = np.empty((T, BFULL, NO), np.float32)
    for t in range(T):
        reset2 = (mem2 > one).astype(np.float32)
        mem2 = (beta * mem2 + cur2[t]) * (one - reset2)
        spk2_rec[t] = (mem2 > one).astype(np.float32)
        mem2_rec[t] = mem2
    return (x, spk1_rec, spk2_rec, mem2_rec)


# revision 24
# speedup vs baseline: 1.0256x; 1.0256x over previous
"""Trainium2 Bass kernel for a 2-layer spiking net (LIF, zero reset).

Computation (T=100 steps, B=256, 784 -> 1024 -> 10):
    cur1_t  = relu(x_t @ w1.T)
    mem1_t  = (beta*mem1 + cur1_t) * (1 - (mem1_prev > th1))
    spk1_t  = (mem1_t > th1)
    cur2_t  = relu(spk1_t @ w2.T)
    mem2_t  = (beta*mem2 + cur2_t) * (1 - (mem2_prev > 1))
    spk2_t  = (mem2_t > 1)
returns (x, spk1_rec, spk2_rec, mem2_rec)

Device strategy (8 cores, data-parallel over batch, 32 rows each):
  - normalize layer-1 state by threshold: y = mem1/th1 by folding 1/th1 into
    w1 rows on the host; spike test becomes y > 1.0 (cheap tensor_scalar).
  - phase A (PE): cur'_t for ALL t in one batched matmul (time-independent).
    fp32 matmul on trn2 runs ~3.7 cycles/col (LOW_HIGH dual pass), so the
    fp32 product is computed as a 3-term 16-bit split at 1 cycle/col:
        x@w ~= xh@wh + xh@wl8 + xl8@wh
    with xh=fp16(x), wh=fp16(w), wl8=bf16(w-wh), xl8=bf16(x-xh).
    bf16 for the correction terms avoids fp16 subnormal flushing; residual
    error ~2^-19 relative, comparable to fp32 rounding for this use.
    The K=16 leftover input dims of all three terms are merged into one
    K=48 bf16 "tail pack" chunk, so each psum group is 19 passes.  Pairs of
    N=256 matmul halves share one weight load.
  - phase B (DVE): 3 ops/step: u=(y*beta)+c', y=u*n_prev, n=(y<=1).
    n (bf16, exact for 0/1) is the complement of the spike train and is the
    only large output; spk = 1-n recovered on host.
  - layer 2 runs on the host: cur2 = relu(spk1 @ w2.T) is 0.26% of the
    model FLOPs (BLAS sgemm, same summation class as the reference), and
    the [256,10] membrane scan is bit-identical fp32 numpy.
"""

import sys

if "/opt/trn_rl_repo" not in sys.path:
    sys.path.insert(0, "/opt/trn_rl_repo")

import numpy as np
import ml_dtypes

BF16 = ml_dtypes.bfloat16

T, BFULL, NI, NH, NO = 100, 256, 784, 1024, 10
NCORES = 8
B = BFULL // NCORES          # 32 batch rows per core
TB = T * B                   # 3200 (t*B + b) columns per core
BETA = 0.95
KC, KP = 6, 128              # full contraction chunks over NI (dims 0..767)
KT = 48                      # tail pack: 3x16 leftover dims as one bf16 chunk
NIF = KC * KP                # 768
MC = NH // 128               # 8 m-chunks over NH
W2C = 16                     # w2 hi/lo K-chunks (2*1024/128)
SPB = 16                     # max steps per block (16*32 = 512 psum cols)
# small first blocks so the sequential DVE scan starts before all inputs
# land; small last blocks so little DVE work trails the final matmul
BLOCKS = [(0, 8), (8, 8), (16, 16), (32, 16), (48, 16), (64, 16),
          (80, 12), (92, 4), (96, 4)]
assert sum(b[1] for b in BLOCKS) == T

_CACHE = {}


def _build_nc():
    import concourse.bass as bass
    import concourse.mybir as mybir
    import concourse.tile as tile
    from concourse import bacc

    dt = mybir.dt.float32
    dth = mybir.dt.float16
    dtb = mybir.dt.bfloat16
    AL = mybir.AluOpType
    AF = mybir.ActivationFunctionType

    nc = bacc.Bacc(
        "TRN2", target_bir_lowering=False, debug=False, enable_asserts=True
    )
    xh_d = nc.declare_dram_parameter("xh", [NIF, TB], dth, isOutput=False)
    xl_d = nc.declare_dram_parameter("xl8", [NIF, TB], dtb, isOutput=False)
    xt_d = nc.declare_dram_parameter("xtp", [KT, TB], dtb, isOutput=False)
    w1h_d = nc.declare_dram_parameter("w1h", [NIF, NH], dth, isOutput=False)
    w1l_d = nc.declare_dram_parameter("w1l8", [NIF, NH], dtb, isOutput=False)
    wt_d = nc.declare_dram_parameter("wtp", [KT, NH], dtb, isOutput=False)
    n_d = nc.declare_dram_parameter("n_rec", [128, T, MC * B], dtb, isOutput=True)

    with tile.TileContext(nc) as tc:
        with (
            tc.tile_pool(name="xs", bufs=1) as xs_pool,
            tc.tile_pool(name="w", bufs=1) as w_pool,
            tc.tile_pool(name="cbuf", bufs=2) as c_pool,
            tc.tile_pool(name="nbuf", bufs=4) as n_pool,
            tc.tile_pool(name="state", bufs=2) as st_pool,
            tc.tile_pool(name="u", bufs=2) as u_pool,
            tc.tile_pool(name="psA", bufs=4, space=bass.MemorySpace.PSUM) as psA,
        ):
            # ---- weights + first blocks of x, spread over two DMA paths.
            # nc.sync is HWDGE; nc.gpsimd is SWDGE on the otherwise-idle
            # GpSimd engine (issuing from nc.scalar would block the phase-A
            # relus queued behind the DMAs on the ACT ring).
            rings = [nc.sync, nc.sync]

            def dma(i, dst, src):
                rings[i % 2].dma_start(dst, src)

            w1h_sb, w1l_sb, xh_sb, xl_sb = [], [], [], []
            wt_sb = w_pool.tile([KT, NH], dtb, tag="wtp")
            dma(0, wt_sb[:], wt_d[:])
            xt_sb = xs_pool.tile([KT, TB], dtb, tag="xtp")
            dma(1, xt_sb[:, 0:256], xt_d[:, 0:256])
            # weight loads split by m-column half: the first psum groups
            # (m=0..3) only need weight columns 0..511, so phase A starts
            # before the second halves arrive
            for k in range(KC):
                ksl = slice(k * KP, (k + 1) * KP)
                wh = w_pool.tile([KP, NH], dth, tag=f"w1h{k}")
                dma(k, wh[:, 0:512], w1h_d[ksl, 0:512])
                w1h_sb.append(wh)
                wl = w_pool.tile([KP, NH], dtb, tag=f"w1l{k}")
                dma(k + 1, wl[:, 0:512], w1l_d[ksl, 0:512])
                w1l_sb.append(wl)
                xk = xs_pool.tile([KP, TB], dth, tag=f"xh{k}")
                dma(k, xk[:, 0:256], xh_d[ksl, 0:256])
                xh_sb.append(xk)
                xlk = xs_pool.tile([KP, TB], dtb, tag=f"xl{k}")
                dma(k + 1, xlk[:, 0:256], xl_d[ksl, 0:256])
                xl_sb.append(xlk)
            for k in range(KC):
                ksl = slice(k * KP, (k + 1) * KP)
                dma(k, w1h_sb[k][:, 512:NH], w1h_d[ksl, 512:NH])
                dma(k + 1, w1l_sb[k][:, 512:NH], w1l_d[ksl, 512:NH])
            dma(0, xt_sb[:, 256:512], xt_d[:, 256:512])
            for k in range(KC):
                ksl = slice(k * KP, (k + 1) * KP)
                dma(k, xh_sb[k][:, 256:512], xh_d[ksl, 256:512])
                dma(k + 1, xl_sb[k][:, 256:512], xl_d[ksl, 256:512])
            dma(0, xt_sb[:, 512:1024], xt_d[:, 512:1024])
            for k in range(KC):
                ksl = slice(k * KP, (k + 1) * KP)
                dma(k, xh_sb[k][:, 512:1024], xh_d[ksl, 512:1024])
                dma(k + 1, xl_sb[k][:, 512:1024], xl_d[ksl, 512:1024])
            dma(0, xt_sb[:, 1024:TB], xt_d[:, 1024:TB])
            for k in range(KC):
                ksl = slice(k * KP, (k + 1) * KP)
                dma(k, xh_sb[k][:, 1024:TB], xh_d[ksl, 1024:TB])
                dma(k + 1, xl_sb[k][:, 1024:TB], xl_d[ksl, 1024:TB])

            # ---- state init
            y_prev = st_pool.tile([128, MC, B], dt, tag="y")
            nc.vector.memset(y_prev[:], 0.0)
            n_init = st_pool.tile([128, MC, B], dtb, tag="ninit")
            nc.vector.memset(n_init[:], 1.0)
            n_prev = n_init[:]

            cbufs = {}
            nbufs = {}

            def phase_A(j):
                t0, spb = BLOCKS[j]
                ncols = spb * B
                # halves of the psum bank share one LDWEIGHTS (same lhsT
                # back-to-back streams at the no-reload rate)
                if ncols > 256:
                    halves = [(0, 256), (256, ncols)]
                else:
                    halves = [(0, ncols)]
                cb = c_pool.tile([128, MC, SPB, B], dt, tag="cbuf")
                cbufs[j] = cb
                # (lhsT, rhs) chunk pairs: 3-term split over dims 0..767
                # plus the K=48 bf16 tail pack covering dims 768..783
                pairs = (
                    [(w1h_sb[k], xh_sb[k]) for k in range(KC)]
                    + [(w1l_sb[k], xh_sb[k]) for k in range(KC)]
                    + [(w1h_sb[k], xl_sb[k]) for k in range(KC)]
                    + [(wt_sb, xt_sb)]
                )
                for m in range(MC):
                    msl = slice(m * 128, (m + 1) * 128)
                    ps = psA.tile([128, 512], dt, tag="psA")
                    n_mm = len(pairs) * len(halves)
                    i_mm = 0
                    for w_t, x_t in pairs:
                        for h0, h1 in halves:
                            nc.tensor.matmul(
                                ps[:, h0:h1],
                                w_t[:, msl],
                                x_t[:, t0 * B + h0 : t0 * B + h1],
                                start=(i_mm == 0),
                                stop=(i_mm == n_mm - 1),
                                skip_group_check=True,
                            )
                            i_mm += 1
                    nc.scalar.activation(cb[:, m, 0:spb, :], ps[:, 0:ncols], AF.Relu)

            def phase_B(j):
                nonlocal y_prev, n_prev
                t0, spb = BLOCKS[j]
                cb = cbufs[j]
                nb = n_pool.tile([128, SPB, MC, B], dtb, tag="nbuf")
                nbufs[j] = nb
                for s in range(spb):
                    u = u_pool.tile([128, MC, B], dt, tag="u")
                    nc.vector.scalar_tensor_tensor(
                        u[:], y_prev[:], BETA, cb[:, :, s, :], op0=AL.mult, op1=AL.add
                    )
                    y_new = st_pool.tile([128, MC, B], dt, tag="y")
                    nc.vector.tensor_tensor(y_new[:], u[:], n_prev, op=AL.mult)
                    nc.vector.tensor_scalar(
                        nb[:, s], y_new[:], 1.0, None, op0=AL.is_le
                    )
                    y_prev = y_new
                    n_prev = nb[:, s]
                nc.sync.dma_start(n_d[:, t0 : t0 + spb, :], nb[:, 0:spb, :, :])

            # emission order keeps PE one block ahead of the serial DVE chain
            nblocks = len(BLOCKS)
            phase_A(0)
            if nblocks > 1:
                phase_A(1)
            for j in range(nblocks):
                phase_B(j)
                if j + 2 < nblocks:
                    phase_A(j + 2)

    nc.compile()
    return nc


def _get_nc():
    if "nc" not in _CACHE:
        _CACHE["nc"] = _build_nc()
    return _CACHE["nc"]


def kernel(x, w1, w2, th1):
    from concourse.bass_utils import run_bass_kernel_spmd

    x = np.ascontiguousarray(x, dtype=np.float32)
    w1 = np.asarray(w1, dtype=np.float32)
    w2 = np.asarray(w2, dtype=np.float32)
    th1 = np.asarray(th1, dtype=np.float32)

    # fold 1/th1 into w1 rows (best fp32 approximation via f64 divide)
    w1p = (w1.astype(np.float64) / th1.astype(np.float64)[:, None]).astype(np.float32)
    w1t = np.ascontiguousarray(w1p.T)                      # [784, 1024] f32
    w1h = w1t[:NIF].astype(np.float16)
    w1l8 = (w1t[:NIF] - w1h.astype(np.float32)).astype(BF16)
    wtt = w1t[NIF:]                                        # [16, 1024] tail dims
    wth = wtt.astype(BF16)
    wtl = (wtt - wth.astype(np.float32)).astype(BF16)
    wtp = np.ascontiguousarray(np.concatenate([wth, wtl, wth], axis=0))  # [48,1024]
    in_maps = []
    for c in range(NCORES):
        xs = x[:, c * B : (c + 1) * B, :]                  # [100, 32, 784]
        xt = np.ascontiguousarray(xs.reshape(TB, NI).T)    # [784, 3200] f32
        xh = xt[:NIF].astype(np.float16)
        xl8 = (xt[:NIF] - xh.astype(np.float32)).astype(BF16)
        xtt = xt[NIF:]                                     # [16, 3200]
        xth = xtt.astype(BF16)
        xtl = (xtt - xth.astype(np.float32)).astype(BF16)
        # row pairing with wtp: [xth<->wth, xth<->wtl, xtl<->wth]
        xtp = np.ascontiguousarray(np.concatenate([xth, xth, xtl], axis=0))
        in_maps.append(
            {"xh": xh, "xl8": xl8, "xtp": xtp, "w1h": w1h, "w1l8": w1l8,
             "wtp": wtp}
        )

    nc = _get_nc()
    res = run_bass_kernel_spmd(nc, in_maps, list(range(NCORES)))
    _CACHE["last_run"] = res
    results = res.results

    spk1_rec = np.empty((T, BFULL, NH), np.float32)
    for c in range(NCORES):
        n = results[c]["n_rec"].astype(np.float32)         # [128, 100, 256]
        # n[p, t, m*32+b] -> neuron h = m*128+p, batch col c*32+b
        sp = 1.0 - n.reshape(128, T, MC, B).transpose(1, 3, 2, 0)  # [T, B, MC, 128]
        spk1_rec[:, c * B : (c + 1) * B, :] = sp.reshape(T, B, NH)

    # layer-2 currents on host: tiny matmul (0.26% of model FLOPs), and BLAS
    # sgemm matches the reference's own fp32 matmul ordering closely
    cur2 = np.maximum(
        spk1_rec.reshape(T * BFULL, NH) @ w2.T, np.float32(0.0)
    ).reshape(T, BFULL, NO)

    # layer-2 membrane scan on host, fp32 ops mirroring the reference exactly
    beta = np.float32(BETA)
    one = np.float32(1.0)
    mem2 = np.zeros((BFULL, NO), np.float32)
    spk2_rec = np.empty((T, BFULL, NO), np.float32)
    mem2_rec = np.empty((T, BFULL, NO), np.float32)
    for t in range(T):
        reset2 = (mem2 > one).astype(np.float32)
        mem2 = (beta * mem2 + cur2[t]) * (one - reset2)
        spk2_rec[t] = (mem2 > one).astype(np.float32)
        mem2_rec[t] = mem2
    return (x, spk1_rec, spk2_rec, mem2_rec)
